# revision 8
# baseline (speedup 1.0000x reference)
"""Trainium2 Bass kernel for nn_BERTEmbedding_65274912964883.

out[b, l, :] = token_table[seq[b, l]]
             + mean_{g in genres(seq[b, l])} genre_table[g]
             + pos_table[l]

Strategy (8 NeuronCores, SPMD, no collectives):
  - Data-parallel over batch: 256 sequences -> 32 per core (6400 tokens/core).
  - One combined bf16 table [VOCAB, 256] replicated per core: cols 0..127
    token embedding, 128..148 the normalized genre histogram
    (hist[v, g] = count(g)/n_genres(v), a host-side dense re-encoding of the
    ragged genre lists), rest zero. The genre mean reduces on device as
    hist_row @ genre_table per token.
  - The gather is the hard floor: HW indirect DMA fetches exactly one table
    row per partition per instruction (~1.1us of GpSimd per instruction), so
    6400 tokens = 50 instructions ~= 56us. Everything else is built to ride
    UNDER that stream: persistent SBUF tiles (no buffer-recycle stalls), the
    seq load first on its own queue, and a 1-subtile final group so the tail
    after the last gather is short.
  - Compute is d-major (embedding dim on partitions): one XBAR DMA transpose
    per subtile flips the gathered [token, 256] rows into embT|histT slabs.
    histT lands on partitions 0..20, so the genre matmul needs no nonzero
    base partitions (those produce NaN on HW): lhsT = genre_table [21, 128]
    at partition 0, rhs = histT spanning up to 4 subtiles (N=512).
  - token + positional terms are added on DVE (embT + posrotT, then + PSUM),
    downcasting to bf16; positional columns come from a host-prebuilt
    d-major rotated table (28 rotations).
  - Device writes output d-major [128, NSUB, 128] bf16 = out[d, i, p]; host
    un-permutes and upcasts to f32.
"""

import numpy as np
import ml_dtypes

import concourse.bacc as bacc
import concourse.mybir as mybir
import concourse.tile as tile
from concourse.bass import IndirectOffsetOnAxis
from concourse.bass_utils import run_bass_kernel_spmd

VOCAB = 100000
D = 128
G = 21          # genre ids are in [0, 20]
MAXG = 8
CW = 256        # combined-table row: 128 emb + 21 hist + 107 pad (bf16)
B, L = 256, 200
NCORES = 8
BC = B // NCORES          # sequences per core
N = BC * L                # tokens per core (6400)
SUB = 128                 # tokens per subtile (partition dim)
NSUB = N // SUB           # 50
GROUPS = [4] * 11 + [3, 2, 1]   # subtiles per matmul group (sum = NSUB)
NROT = 25                 # distinct values of (128*i) % 200
NROTX = 28                # extended with 3 duplicates so groups never wrap

F32 = mybir.dt.float32
BF16 = mybir.dt.bfloat16
I32 = mybir.dt.int32

assert sum(GROUPS) == NSUB


def emit_core_kernel(tc, seq, ctab, gtab, posrotT, out):
    """Emit the per-core kernel into TileContext `tc`.

    seq     : DRAM [128, NSUB] int32, seq[p, i] = token id of token i*128+p
    ctab    : DRAM [VOCAB, CW] bf16 combined table (emb | hist | pad)
    gtab    : DRAM [G, D] bf16 genre table
    posrotT : DRAM [128, NROTX*128] bf16, posrotT[d, r*128+p] =
              pos_table[(128*r+p) % L, d]
    out     : DRAM [128, NSUB, 128] bf16, out[d, i, p] = emb dim d of token
              i*128+p
    """
    nc = tc.nc
    add = mybir.AluOpType.add

    with (
        tc.tile_pool(name="const", bufs=1) as cpool,
        tc.tile_pool(name="work", bufs=2) as wpool,
        tc.tile_pool(name="psum", bufs=2, space="PSUM") as ppool,
    ):
        # seq first, alone on the sync queue: gathers depend only on it
        seq_sb = cpool.tile([128, NSUB], I32)
        nc.sync.dma_start(out=seq_sb[:], in_=seq)
        # consts ride the scalar queue
        gtab_sb = cpool.tile([G, D], BF16)
        nc.scalar.dma_start(out=gtab_sb[:], in_=gtab)
        posrotT_sb = cpool.tile([128, NROTX * 128], BF16)
        nc.scalar.dma_start(out=posrotT_sb[:], in_=posrotT)

        # persistent full-size intermediates: no buffer recycling anywhere,
        # so the 50 gathers run back-to-back with no wait-for-free stalls.
        cg_sb = cpool.tile([128, NSUB * CW], BF16)      # gathered rows
        cg3 = cg_sb[:].rearrange("p (j c) -> p j c", c=CW)
        xt_sb = cpool.tile([128, NSUB * 256], BF16)     # embT|histT slabs
        xt4 = xt_sb[:].rearrange("p (j s q) -> p j s q", s=2, q=128)

        # the gather stream: one indirect DMA per 128-token subtile
        for j in range(NSUB):
            nc.gpsimd.indirect_dma_start(
                out=cg3[:, j, :],
                out_offset=None,
                in_=ctab,
                in_offset=IndirectOffsetOnAxis(
                    ap=seq_sb[:, j:j + 1], axis=0
                ),
            )

        # compute: XBAR transpose per subtile, then per-group matmul + adds
        i0 = 0
        for gi, ng in enumerate(GROUPS):
            for j in range(ng):
                jj = i0 + j
                eng = nc.sync if jj % 2 == 0 else nc.scalar
                eng.dma_start_transpose(
                    out=xt4[:, jj, :, :],
                    in_=cg_sb[:, jj * CW:(jj + 1) * CW],
                )

            # genre mean for ng subtiles in ONE matmul:
            # PSUM[d, j*128+p] = sum_g gtab[g, d] * histT[g, j, p]
            gm_ps = ppool.tile([128, ng * 128], F32, tag=f"gm{ng}", bufs=2)
            nc.tensor.matmul(
                out=gm_ps[:],
                lhsT=gtab_sb[:],
                rhs=xt4[0:G, i0:i0 + ng, 1, :],
                start=True, stop=True,
                skip_group_check=True,
            )

            # embT + posT on DVE, then + genre mean (PSUM), downcast bf16
            r0 = i0 % NROT
            ep_sb = wpool.tile([128, ng * 128], BF16, tag=f"ep{ng}", bufs=2)
            nc.vector.tensor_tensor(
                out=ep_sb[:].rearrange("p (j q) -> p j q", q=128),
                in0=xt4[:, i0:i0 + ng, 0, :],
                in1=posrotT_sb[:, r0 * 128:(r0 + ng) * 128]
                    .rearrange("p (j q) -> p j q", q=128),
                op=add,
            )
            out_sb = wpool.tile([128, ng * 128], BF16, tag=f"o{ng}", bufs=2)
            nc.vector.tensor_tensor(
                out=out_sb[:], in0=ep_sb[:], in1=gm_ps[:], op=add,
            )
            nc.sync.dma_start(
                out=out[:, i0:i0 + ng, :],
                in_=out_sb[:].rearrange("p (j q) -> p j q", q=128),
            )
            i0 += ng


def build_nc():
    nc = bacc.Bacc("TRN2", target_bir_lowering=False, debug=False)
    seq = nc.dram_tensor("seq", [128, NSUB], I32, kind="ExternalInput").ap()
    ctab = nc.dram_tensor("ctab", [VOCAB, CW], BF16, kind="ExternalInput").ap()
    gtab = nc.dram_tensor("gtab", [G, D], BF16, kind="ExternalInput").ap()
    posrotT = nc.dram_tensor(
        "posrotT", [128, NROTX * 128], BF16, kind="ExternalInput").ap()
    out = nc.dram_tensor("out", [128, NSUB, 128], BF16,
                         kind="ExternalOutput").ap()

    with tile.TileContext(nc) as tc:
        emit_core_kernel(tc, seq, ctab, gtab, posrotT, out)
    nc.compile()
    return nc


_NC_CACHE = None


def _get_nc():
    global _NC_CACHE
    if _NC_CACHE is None:
        _NC_CACHE = build_nc()
    return _NC_CACHE


def make_ctab(token_table, token_genre_ids, genre_counts):
    gids = np.asarray(token_genre_ids).astype(np.int64)      # [V, MAXG]
    cnts = np.asarray(genre_counts).astype(np.int64)         # [V]
    valid = np.arange(MAXG)[None, :] < cnts[:, None]         # [V, MAXG]
    flat = (np.arange(VOCAB)[:, None] * G + gids)[valid]
    hist = np.bincount(flat, minlength=VOCAB * G).reshape(VOCAB, G)
    histn = hist.astype(np.float32) / cnts[:, None].astype(np.float32)

    ctab = np.zeros((VOCAB, CW), dtype=ml_dtypes.bfloat16)
    ctab[:, 0:D] = np.asarray(token_table, dtype=np.float32).astype(
        ml_dtypes.bfloat16)
    ctab[:, D:D + G] = histn.astype(ml_dtypes.bfloat16)
    return ctab


def make_posrotT(pos_table):
    pos = np.asarray(pos_table, dtype=np.float32)
    pr = np.zeros((128, NROTX * 128), dtype=np.float32)
    p = np.arange(128)
    for r in range(NROTX):
        pr[:, r * 128:(r + 1) * 128] = pos[(128 * r + p) % L, :].T
    return pr.astype(ml_dtypes.bfloat16)


def prep_host_inputs(sequence, token_table, genre_table, pos_table,
                     token_genre_ids, genre_counts):
    """Host-side sharding / layout prep. Returns in_maps for the 8 cores."""
    seq = np.ascontiguousarray(np.asarray(sequence).astype(np.int32)).reshape(B, L)
    ctab = make_ctab(token_table, token_genre_ids, genre_counts)
    gtab = np.asarray(genre_table, dtype=np.float32)[:G].astype(
        ml_dtypes.bfloat16)
    posrotT = make_posrotT(pos_table)

    in_maps = []
    for c in range(NCORES):
        seq_core = seq[c * BC:(c + 1) * BC].reshape(N)
        # device layout: seq_dev[p, i] = seq_core[i*128 + p]
        seq_dev = np.ascontiguousarray(seq_core.reshape(NSUB, 128).T)
        in_maps.append({
            "seq": seq_dev,
            "ctab": ctab,
            "gtab": gtab,
            "posrotT": posrotT,
        })
    return in_maps


def postprocess(results):
    """Un-permute per-core outputs, upcast to f32, concatenate to [B, L, D]."""
    outs = []
    for c in range(NCORES):
        o = np.asarray(results[c]["out"])  # [128, NSUB, 128] = [d, i, p]
        outs.append(o.transpose(1, 2, 0).reshape(BC, L, D))
    return np.concatenate(outs, axis=0).astype(np.float32)


def kernel(sequence, token_table, genre_table, pos_table, token_genre_ids,
           genre_counts):
    nc = _get_nc()
    in_maps = prep_host_inputs(sequence, token_table, genre_table, pos_table,
                               token_genre_ids, genre_counts)
    res = run_bass_kernel_spmd(nc, in_maps, core_ids=list(range(NCORES)))
    return postprocess(res.results)


# revision 10
# speedup vs baseline: 1.0075x; 1.0075x over previous
"""Trainium2 Bass kernel for nn_BERTEmbedding_65274912964883.

out[b, l, :] = token_table[seq[b, l]]
             + mean_{g in genres(seq[b, l])} genre_table[g]
             + pos_table[l]

Strategy (8 NeuronCores, SPMD, no collectives):
  - Data-parallel over batch: 256 sequences -> 32 per core (6400 tokens/core).
  - One combined bf16 table [VOCAB, 256] replicated per core: cols 0..127
    token embedding, 128..148 the normalized genre histogram
    (hist[v, g] = count(g)/n_genres(v), a host-side dense re-encoding of the
    ragged genre lists), rest zero. The genre mean reduces on device as
    hist_row @ genre_table per token.
  - The gather is the hard floor: HW indirect DMA fetches exactly one table
    row per partition per instruction (~1.1us of GpSimd per instruction), so
    6400 tokens = 50 instructions ~= 56us. Everything else is built to ride
    UNDER that stream: persistent SBUF tiles (no buffer-recycle stalls), the
    seq load first on its own queue, and a 1-subtile final group so the tail
    after the last gather is short.
  - Compute is d-major (embedding dim on partitions): one XBAR DMA transpose
    per subtile flips the gathered [token, 256] rows into embT|histT slabs.
    histT lands on partitions 0..20, so the genre matmul needs no nonzero
    base partitions (those produce NaN on HW): lhsT = genre_table [21, 128]
    at partition 0, rhs = histT spanning up to 4 subtiles (N=512).
  - token + positional terms are added on DVE (embT + posrotT, then + PSUM),
    downcasting to bf16; positional columns come from a host-prebuilt
    d-major rotated table (28 rotations).
  - Device writes output d-major [128, NSUB, 128] bf16 = out[d, i, p]; host
    un-permutes and upcasts to f32.
"""

import numpy as np
import ml_dtypes

import concourse.bacc as bacc
import concourse.mybir as mybir
import concourse.tile as tile
from concourse.bass import IndirectOffsetOnAxis
from concourse.bass_utils import run_bass_kernel_spmd

VOCAB = 100000
D = 128
G = 21          # genre ids are in [0, 20]
MAXG = 8
CW = 256        # combined-table row: 128 emb + 21 hist + 107 pad (bf16)
B, L = 256, 200
NCORES = 8
BC = B // NCORES          # sequences per core
N = BC * L                # tokens per core (6400)
SUB = 128                 # tokens per subtile (partition dim)
NSUB = N // SUB           # 50
GROUPS = [4] * 11 + [3, 2, 1]   # subtiles per matmul group (sum = NSUB)
NROT = 25                 # distinct values of (128*i) % 200
NROTX = 28                # extended with 3 duplicates so groups never wrap

F32 = mybir.dt.float32
BF16 = mybir.dt.bfloat16
I32 = mybir.dt.int32

assert sum(GROUPS) == NSUB


def emit_core_kernel(tc, seq, ctab, gtab, posrotT, out):
    """Emit the per-core kernel into TileContext `tc`.

    seq     : DRAM [128, NSUB] int32, seq[p, i] = token id of token i*128+p
    ctab    : DRAM [VOCAB, CW] bf16 combined table (emb | hist | pad)
    gtab    : DRAM [G, D] bf16 genre table
    posrotT : DRAM [128, NROTX*128] bf16, posrotT[d, r*128+p] =
              pos_table[(128*r+p) % L, d]
    out     : DRAM [128, NSUB, 128] bf16, out[d, i, p] = emb dim d of token
              i*128+p
    """
    nc = tc.nc
    add = mybir.AluOpType.add

    with (
        tc.tile_pool(name="const", bufs=1) as cpool,
        tc.tile_pool(name="work", bufs=2) as wpool,
        tc.tile_pool(name="psum", bufs=2, space="PSUM") as ppool,
    ):
        # seq first, alone on the sync queue: gathers depend only on it
        seq_sb = cpool.tile([128, NSUB], I32)
        nc.sync.dma_start(out=seq_sb[:], in_=seq)
        # consts ride the scalar queue
        gtab_sb = cpool.tile([G, D], BF16)
        nc.scalar.dma_start(out=gtab_sb[:], in_=gtab)
        posrotT_sb = cpool.tile([128, NROTX * 128], BF16)
        nc.scalar.dma_start(out=posrotT_sb[:], in_=posrotT)

        # persistent per-subtile gather tiles: gather j and its XBAR reader
        # touch only tile j, so the 50 gathers free-run with no false
        # write-after-read hazards against the transposes.
        cg_tiles = [cpool.tile([128, CW], BF16, name=f"cg{j}")
                    for j in range(NSUB)]

        # the gather stream: one indirect DMA per 128-token subtile
        for j in range(NSUB):
            nc.gpsimd.indirect_dma_start(
                out=cg_tiles[j][:],
                out_offset=None,
                in_=ctab,
                in_offset=IndirectOffsetOnAxis(
                    ap=seq_sb[:, j:j + 1], axis=0
                ),
            )

        # compute: XBAR transpose per subtile, then per-group matmul + adds
        i0 = 0
        for gi, ng in enumerate(GROUPS):
            xt_sb = cpool.tile([128, ng * 256], BF16, name=f"xt{gi}")
            xt4 = xt_sb[:].rearrange("p (j s q) -> p j s q", s=2, q=128)
            for j in range(ng):
                jj = i0 + j
                eng = nc.sync if jj % 2 == 0 else nc.scalar
                eng.dma_start_transpose(
                    out=xt4[:, j, :, :],
                    in_=cg_tiles[jj][:],
                )

            # genre mean for ng subtiles in ONE matmul:
            # PSUM[d, j*128+p] = sum_g gtab[g, d] * histT[g, j, p]
            gm_ps = ppool.tile([128, ng * 128], F32, tag=f"gm{ng}", bufs=2)
            nc.tensor.matmul(
                out=gm_ps[:],
                lhsT=gtab_sb[:],
                rhs=xt4[0:G, :, 1, :],
                start=True, stop=True,
                skip_group_check=True,
            )

            # embT + posT on DVE, then + genre mean (PSUM), downcast bf16
            r0 = i0 % NROT
            ep_sb = wpool.tile([128, ng * 128], BF16, tag=f"ep{ng}", bufs=2)
            nc.vector.tensor_tensor(
                out=ep_sb[:].rearrange("p (j q) -> p j q", q=128),
                in0=xt4[:, :, 0, :],
                in1=posrotT_sb[:, r0 * 128:(r0 + ng) * 128]
                    .rearrange("p (j q) -> p j q", q=128),
                op=add,
            )
            out_sb = wpool.tile([128, ng * 128], BF16, tag=f"o{ng}", bufs=2)
            nc.vector.tensor_tensor(
                out=out_sb[:], in0=ep_sb[:], in1=gm_ps[:], op=add,
            )
            nc.sync.dma_start(
                out=out[:, i0:i0 + ng, :],
                in_=out_sb[:].rearrange("p (j q) -> p j q", q=128),
            )
            i0 += ng


def build_nc():
    nc = bacc.Bacc("TRN2", target_bir_lowering=False, debug=False)
    seq = nc.dram_tensor("seq", [128, NSUB], I32, kind="ExternalInput").ap()
    ctab = nc.dram_tensor("ctab", [VOCAB, CW], BF16, kind="ExternalInput").ap()
    gtab = nc.dram_tensor("gtab", [G, D], BF16, kind="ExternalInput").ap()
    posrotT = nc.dram_tensor(
        "posrotT", [128, NROTX * 128], BF16, kind="ExternalInput").ap()
    out = nc.dram_tensor("out", [128, NSUB, 128], BF16,
                         kind="ExternalOutput").ap()

    with tile.TileContext(nc) as tc:
        emit_core_kernel(tc, seq, ctab, gtab, posrotT, out)
    nc.compile()
    return nc


_NC_CACHE = None


def _get_nc():
    global _NC_CACHE
    if _NC_CACHE is None:
        _NC_CACHE = build_nc()
    return _NC_CACHE


def make_ctab(token_table, token_genre_ids, genre_counts):
    gids = np.asarray(token_genre_ids).astype(np.int64)      # [V, MAXG]
    cnts = np.asarray(genre_counts).astype(np.int64)         # [V]
    valid = np.arange(MAXG)[None, :] < cnts[:, None]         # [V, MAXG]
    flat = (np.arange(VOCAB)[:, None] * G + gids)[valid]
    hist = np.bincount(flat, minlength=VOCAB * G).reshape(VOCAB, G)
    histn = hist.astype(np.float32) / cnts[:, None].astype(np.float32)

    ctab = np.zeros((VOCAB, CW), dtype=ml_dtypes.bfloat16)
    ctab[:, 0:D] = np.asarray(token_table, dtype=np.float32).astype(
        ml_dtypes.bfloat16)
    ctab[:, D:D + G] = histn.astype(ml_dtypes.bfloat16)
    return ctab


def make_posrotT(pos_table):
    pos = np.asarray(pos_table, dtype=np.float32)
    pr = np.zeros((128, NROTX * 128), dtype=np.float32)
    p = np.arange(128)
    for r in range(NROTX):
        pr[:, r * 128:(r + 1) * 128] = pos[(128 * r + p) % L, :].T
    return pr.astype(ml_dtypes.bfloat16)


def prep_host_inputs(sequence, token_table, genre_table, pos_table,
                     token_genre_ids, genre_counts):
    """Host-side sharding / layout prep. Returns in_maps for the 8 cores."""
    seq = np.ascontiguousarray(np.asarray(sequence).astype(np.int32)).reshape(B, L)
    ctab = make_ctab(token_table, token_genre_ids, genre_counts)
    gtab = np.asarray(genre_table, dtype=np.float32)[:G].astype(
        ml_dtypes.bfloat16)
    posrotT = make_posrotT(pos_table)

    in_maps = []
    for c in range(NCORES):
        seq_core = seq[c * BC:(c + 1) * BC].reshape(N)
        # device layout: seq_dev[p, i] = seq_core[i*128 + p]
        seq_dev = np.ascontiguousarray(seq_core.reshape(NSUB, 128).T)
        in_maps.append({
            "seq": seq_dev,
            "ctab": ctab,
            "gtab": gtab,
            "posrotT": posrotT,
        })
    return in_maps


def postprocess(results):
    """Un-permute per-core outputs, upcast to f32, concatenate to [B, L, D]."""
    outs = []
    for c in range(NCORES):
        o = np.asarray(results[c]["out"])  # [128, NSUB, 128] = [d, i, p]
        outs.append(o.transpose(1, 2, 0).reshape(BC, L, D))
    return np.concatenate(outs, axis=0).astype(np.float32)


def kernel(sequence, token_table, genre_table, pos_table, token_genre_ids,
           genre_counts):
    nc = _get_nc()
    in_maps = prep_host_inputs(sequence, token_table, genre_table, pos_table,
                               token_genre_ids, genre_counts)
    res = run_bass_kernel_spmd(nc, in_maps, core_ids=list(range(NCORES)))
    return postprocess(res.results)
